# revision 21
# baseline (speedup 1.0000x reference)
"""Trainium2 Bass kernel for quantized int8 per-channel Conv2d.

Reference semantics (fp32):
  x_f = (x_int8 - 7) * 0.01
  w_f = (w_int8 - zp[cout]) * scale[cout]
  y   = round(conv2d_valid(x_f, w_f) + bias[cout])  -> int32

Algorithm: 1D Winograd along the HEIGHT axis (width taps direct),
ALTERNATING per image between F(2,3) (4 points, cheap transforms) and
F(4,3) (6 points, 2x less PE work, heavier transforms).  The mix
balances the two bottleneck engines: PE ~108us, DVE ~95us per core.
Row-tiling keeps the innermost (width) axis dense so every transform
runs in the DVE's 2x packed fp16 mode with no strided "deal" copies.

F(2,3):  V = [r0-r2, r1+r2, r2-r1, r1-r3]  (27 row-pairs, exact)
         y0 = m0+m1+m2, y1 = m1-m2-m3
F(4,3):  14 quads of 4 rows (input rows up to 57: 2 zero pad rows)
         b0=4(d0-d2)+(d4-d2) b1=-4(d1+d2)+(d3+d4) b2=4(d1-d2)+(d4-d3)
         b3=2e+f b4=-2e+f b5=-4e+(d5-d3)   [e=d3-d1, f=d4-d2]
         A^T=[[1,1,1,1,1,0],[0,1,-1,2,-2,0],[0,1,1,4,4,0],[0,1,-1,8,-8,1]]

U = G*(w-zp)*0.01*scale in fp16; the (x-7) zeropoint folds into the
bias, which rides the j=1 drain (m1's A^T column is all ones).  Output
rounding uses the engines' native fp32->int32 RNE conversion (verified
on HW): the final combines write int32 directly.

Engines: PE j-major matmuls (j=1 first); ACT casts int8->fp16 and
drains PSUM; DVE transforms + A^T combines.  GPSIMD idle (slow SBUF
path + steals the DVE port).  All DMA via sync queue (HWDGE).  Next
image's transform groups are emitted one per combine slot to avoid
bursty DVE queues.  The last (n,m) fuses the j=0 drain+combine (TT
reads PSUM) per chunk.  Sharding: batch 32 over 8 cores.
"""

import numpy as np

import concourse.bass as bass
import concourse.mybir as mybir
from concourse import bacc
from concourse.tile import TileContext
from concourse.bass_utils import run_bass_kernel_spmd

N, CIN, H, W = 32, 256, 56, 56
COUT, KH, KW = 256, 3, 3
HO, WO = H - KH + 1, W - KW + 1          # 54, 54
NCORES = 8
NPER = N // NCORES
HW = H * W
XPAD = HW + 64
KT = CIN // 128
MT = COUT // 128
XR = 60                                   # padded rows in fp16 x buffer

NJ4 = 6                                   # F(4,3) points
NQ4 = 14                                  # row quads
QCH4 = 7                                  # quads per chunk
NCH4 = 2
J4 = (1, 2, 3, 4, 5, 0)

NJ2 = 4                                   # F(2,3) points
NQ2 = 27                                  # row pairs
QCH2 = 9                                  # pairs per chunk
NCH2 = 3
J2 = (1, 2, 3, 0)

KINDS = (2, 4, 4, 4)                      # per-image variant

G4 = np.array([
    [1 / 4, 0, 0],
    [-1 / 6, -1 / 6, -1 / 6],
    [-1 / 6, 1 / 6, -1 / 6],
    [1 / 24, 1 / 12, 1 / 6],
    [1 / 24, -1 / 12, 1 / 6],
    [0, 0, 1],
], dtype=np.float64)
G2 = np.array([
    [1, 0, 0],
    [0.5, 0.5, 0.5],
    [0.5, -0.5, 0.5],
    [0, 0, 1],
], dtype=np.float64)

_CACHE = {}


def _build_program():
    nc = bacc.Bacc("TRN2", target_bir_lowering=False, debug=False,
                   num_devices=NCORES)
    dt = mybir.dt
    f16 = dt.float16
    AF = mybir.ActivationFunctionType
    ALU = mybir.AluOpType

    x_d = nc.dram_tensor("x", [NPER, CIN, H, W], dt.int8, kind="ExternalInput")
    u4_d = nc.dram_tensor("u4", [NJ4, 128, KT, KW, MT, 128], f16,
                          kind="ExternalInput")
    u2_d = nc.dram_tensor("u2", [NJ2, 128, KT, KW, MT, 128], f16,
                          kind="ExternalInput")
    b2_d = nc.dram_tensor("bias2", [COUT], dt.float32, kind="ExternalInput")
    out_d = nc.dram_tensor("out", [NPER, COUT, HO, WO], dt.int32,
                           kind="ExternalOutput")

    with TileContext(nc) as tc:
        with (
            tc.tile_pool(name="const", bufs=1) as cpool,
            tc.tile_pool(name="xin", bufs=2) as xpool,
            tc.tile_pool(name="xf16", bufs=1) as fpool,
            tc.tile_pool(name="v4", bufs=2) as v4pool,
            tc.tile_pool(name="v2", bufs=1) as v2pool,
            tc.tile_pool(name="tsc", bufs=4) as spool,
            tc.tile_pool(name="psum", bufs=7, space="PSUM") as ppool,
            tc.tile_pool(name="m4", bufs=2) as m4pool,
            tc.tile_pool(name="m2", bufs=2) as m2pool,
            tc.tile_pool(name="csc", bufs=1) as epool,
            tc.tile_pool(name="ob4", bufs=1) as o4pool,
            tc.tile_pool(name="ob2", bufs=1) as o2pool,
        ):
            u4sb = cpool.tile([128, NJ4, KT, KW, MT, 128], f16)
            u2sb = cpool.tile([128, NJ2, KT, KW, MT, 128], f16)
            b2 = cpool.tile([128, MT], dt.float32)

            wupw = cpool.tile([128, 128], f16)
            nc.vector.memset(wupw[:, :], 1.0)
            wupx = cpool.tile([128, 512], f16)
            nc.vector.memset(wupx[:, :], 1.0)
            wups = ppool.tile([128, 512], dt.float32, name="wups", tag="wup",
                              bufs=1)
            for _ in range(10):
                nc.tensor.matmul(wups[:, :], wupw[:, :], wupx[:, :],
                                 start=True, stop=True)

            def keepers(rhs, count):
                for _ in range(count):
                    nc.tensor.matmul(wups[:, 0:rhs.shape[-1]], wupw[:, :],
                                     rhs, start=True, stop=True)

            def xdma(n, xb):
                for k in range(KT):
                    nc.sync.dma_start(
                        out=xb[:, k, 0:HW],
                        in_=x_d[n, k * 128:(k + 1) * 128].rearrange(
                            "p h w -> p (h w)"))

            def cast(eng, xb, xf, k):
                dst = xf[:, k].rearrange("p r w -> p (r w)")[:, 0:HW]
                if eng is nc.scalar:
                    eng.copy(dst, xb[:, k, 0:HW])
                else:
                    eng.tensor_copy(dst, xb[:, k, 0:HW])

            TT = nc.vector.tensor_tensor
            STT = nc.vector.scalar_tensor_tensor

            # ---- F(4,3) transforms: merged-k groups (7 emission slots) --
            ts = {}

            def d4v(xf, s):
                xq = xf[:, :, :, :].rearrange("p k (q f) w -> p k q f w",
                                              f=4)
                if s < 4:
                    return xq[:, :, 0:NQ4, s]
                return xq[:, :, 1:NQ4 + 1, s - 4]

            def sc4(name):
                t = spool.tile([128, KT, NQ4, W], f16, name="ts")
                ts[name] = t
                return t[:, :, :, :]

            def g4(name):
                return ts[name][:, :, :, :]

            def prep4(xb, xf, vt):
                d = lambda s: d4v(xf, s)

                def g_cast():
                    nc.vector.memset(xf[:, :, H:H + 2, :], 0.0)
                    cast(nc.scalar, xb, xf, 0)
                    cast(nc.scalar, xb, xf, 1)

                def g_j1():
                    TT(sc4("p1"), d(1), d(2), ALU.add)
                    TT(sc4("p3"), d(3), d(4), ALU.add)
                    STT(vt[:, 1], g4("p1"), -4.0, g4("p3"),
                        ALU.mult, ALU.add)

                def g_j2():
                    TT(sc4("m1"), d(1), d(2), ALU.subtract)
                    TT(sc4("m3"), d(4), d(3), ALU.subtract)
                    STT(vt[:, 2], g4("m1"), 4.0, g4("m3"),
                        ALU.mult, ALU.add)

                def g_j3():
                    TT(sc4("e"), d(3), d(1), ALU.subtract)
                    TT(sc4("f"), d(4), d(2), ALU.subtract)
                    STT(vt[:, 3], g4("e"), 2.0, g4("f"), ALU.mult, ALU.add)

                def g_j4():
                    STT(vt[:, 4], g4("e"), -2.0, g4("f"), ALU.mult, ALU.add)

                def g_j5():
                    TT(sc4("u2"), d(5), d(3), ALU.subtract)
                    STT(vt[:, 5], g4("e"), -4.0, g4("u2"),
                        ALU.mult, ALU.add)

                def g_j0():
                    TT(sc4("u1"), d(0), d(2), ALU.subtract)
                    STT(vt[:, 0], g4("u1"), 4.0, g4("f"), ALU.mult, ALU.add)

                return [g_cast, g_j1, g_j2, g_j3, g_j4, g_j5, g_j0]

            # ---- F(2,3) transforms: merged-k, single-TT points ----------
            def r2v(xf, s):
                xq = xf[:, :, :, :].rearrange("p k (t f) w -> p k t f w",
                                              f=2)
                if s < 2:
                    return xq[:, :, 0:NQ2, s]
                return xq[:, :, 1:NQ2 + 1, s - 2]

            def prep2(xb, xf, vt):
                d = lambda s: r2v(xf, s)

                def g_cast():
                    cast(nc.scalar, xb, xf, 0)
                    cast(nc.scalar, xb, xf, 1)

                def g_j1():
                    TT(vt[:, 1], d(1), d(2), ALU.add)

                def g_j2():
                    TT(vt[:, 2], d(2), d(1), ALU.subtract)

                def g_j3():
                    TT(vt[:, 3], d(1), d(3), ALU.subtract)

                def g_j0():
                    TT(vt[:, 0], d(0), d(2), ALU.subtract)

                return [g_cast, g_j1, g_j2, g_j3, g_j0]

            # ---- startup DMAs: x image 0 first, then U (j=1 first) ------
            xb0 = xpool.tile([128, KT, XPAD], dt.int8, name="xb")
            xf0 = fpool.tile([128, KT, XR, W], f16, name="xf")
            vt20 = v2pool.tile([128, NJ2, KT, NQ2, W], f16, name="vt2")
            xdma(0, xb0)
            nc.sync.dma_start(out=u2sb[:, 1], in_=u2_d[1])
            nc.sync.dma_start(out=b2[:, :],
                              in_=b2_d.rearrange("(m p) -> p m", p=128))
            for j in (2, 3, 0):
                nc.sync.dma_start(out=u2sb[:, j], in_=u2_d[j])
            for j in J4:
                nc.sync.dma_start(out=u4sb[:, j], in_=u4_d[j])

            # ---- image 0 (F23) prologue: split-k for latency ------------
            cast(nc.vector, xb0, xf0, 0)
            cast(nc.scalar, xb0, xf0, 1)
            for j in (1, 2, 3, 0):
                a, b, op = {1: (1, 2, ALU.add), 2: (2, 1, ALU.subtract),
                            3: (1, 3, ALU.subtract),
                            0: (0, 2, ALU.subtract)}[j]
                for k in range(KT):
                    TT(vt20[:, j, k], r2v(xf0, a)[:, k], r2v(xf0, b)[:, k],
                       op)
            keepers(vt20[:, 1, 0].rearrange("p q w -> p (q w)")[:, 0:512],
                    8)

            xfs = {0: xf0}
            vts = {0: vt20}
            preps = []

            for n in range(NPER):
                kind = KINDS[n]
                vt = vts[n]
                last_img = n == NPER - 1
                if not last_img:
                    nkind = KINDS[n + 1]
                    xbn = xpool.tile([128, KT, XPAD], dt.int8, name="xb")
                    xfn = fpool.tile([128, KT, XR, W], f16, name="xf")
                    if nkind == 4:
                        vtn = v4pool.tile([128, NJ4, KT, NQ4, W], f16,
                                          name="vt4")
                        preps = prep4(xbn, xfn, vtn)
                    else:
                        vtn = v2pool.tile([128, NJ2, KT, NQ2, W], f16,
                                          name="vt2")
                        preps = prep2(xbn, xfn, vtn)
                    xdma(n + 1, xbn)
                    xfs[n + 1] = xfn
                    vts[n + 1] = vtn
                else:
                    preps = []

                jorder = J4 if kind == 4 else J2
                nch = NCH4 if kind == 4 else NCH2
                qch = QCH4 if kind == 4 else QCH2
                nq = NQ4 if kind == 4 else NQ2
                usb = u4sb if kind == 4 else u2sb

                for m in range(MT):
                    last = last_img and m == MT - 1
                    if kind == 4:
                        msb = m4pool.tile([128, NJ4, NQ4, WO], f16,
                                          name="msb4")
                        ob = o4pool.tile([128, H, WO], dt.int32, name="ob4")
                        obq = ob[:, :, :].rearrange(
                            "p (q f) w -> p q f w", f=4)
                    else:
                        msb = m2pool.tile([128, NJ2, NQ2, WO], f16,
                                          name="msb2", bufs=1)
                        ob = o2pool.tile([128, NQ2, 2, WO], dt.int32,
                                         name="ob2")
                    M = [msb[:, j] for j in range(len(jorder))]
                    s12 = d12 = s34 = d34 = t0 = u8 = None
                    for j in jorder:
                        ps = [ppool.tile([128, qch, WO], dt.float32,
                                         name="ps", tag="ps")
                              for _ in range(nch)]
                        for k in range(KT):
                            for c in range(KW):
                                lhsT = usb[:, j, k, c, m]
                                for ch in range(nch):
                                    nc.tensor.matmul(
                                        ps[ch][:, :, :], lhsT,
                                        vt[:, j, k, qch * ch:qch * (ch + 1),
                                           c:c + WO],
                                        start=(c == 0 and k == 0),
                                        stop=(c == KW - 1 and k == KT - 1))
                        if not (last and j == 0):
                            for ch in range(nch):
                                dst = msb[:, j, qch * ch:qch * (ch + 1)]
                                if j == 1:
                                    nc.scalar.activation(
                                        dst, ps[ch][:, :, :], AF.Identity,
                                        bias=b2[:, m:m + 1], scale=1.0)
                                else:
                                    nc.scalar.activation(
                                        dst, ps[ch][:, :, :], AF.Copy)
                        # ---- combines (RNE int32 writes) + prep slots ---
                        if kind == 4:
                            if j == 2:
                                s12 = epool.tile([128, NQ4, WO], f16,
                                                 name="c0")
                                d12 = epool.tile([128, NQ4, WO], f16,
                                                 name="c1")
                                TT(s12[:, :, :], M[1], M[2], ALU.add)
                                TT(d12[:, :, :], M[1], M[2], ALU.subtract)
                            elif j == 4:
                                s34 = epool.tile([128, NQ4, WO], f16,
                                                 name="c2")
                                d34 = epool.tile([128, NQ4, WO], f16,
                                                 name="c3")
                                t0 = epool.tile([128, NQ4, WO], f16,
                                                name="c4")
                                u8 = epool.tile([128, NQ4, WO], f16,
                                                name="c5")
                                TT(s34[:, :, :], M[3], M[4], ALU.add)
                                TT(d34[:, :, :], M[3], M[4], ALU.subtract)
                                STT(obq[:, :, 1, :], d34[:, :, :], 2.0,
                                    d12[:, :, :], ALU.mult, ALU.add)
                                STT(obq[:, :, 2, :], s34[:, :, :], 4.0,
                                    s12[:, :, :], ALU.mult, ALU.add)
                                TT(t0[:, :, :], s12[:, :, :], s34[:, :, :],
                                   ALU.add)
                                STT(u8[:, :, :], d34[:, :, :], 8.0,
                                    d12[:, :, :], ALU.mult, ALU.add)
                            elif j == 5:
                                if not last:
                                    TT(obq[:, :, 3, :], u8[:, :, :], M[5],
                                       ALU.add)
                                else:
                                    for ch in range(NCH4):
                                        qs = slice(QCH4 * ch,
                                                   QCH4 * (ch + 1))
                                        TT(obq[:, qs, 3, :], u8[:, qs, :],
                                           msb[:, 5, qs], ALU.add)
                            elif j == 0:
                                if not last:
                                    TT(obq[:, :, 0, :], t0[:, :, :], M[0],
                                       ALU.add)
                                    nc.sync.dma_start(
                                        out=out_d[n, m * 128:(m + 1) * 128],
                                        in_=ob[:, 0:HO, :])
                                else:
                                    for ch in range(NCH4):
                                        qs = slice(QCH4 * ch,
                                                   QCH4 * (ch + 1))
                                        TT(obq[:, qs, 0, :], t0[:, qs, :],
                                           ps[ch][:, :, :], ALU.add)
                                        r0 = 4 * QCH4 * ch
                                        r1 = min(4 * QCH4 * (ch + 1), HO)
                                        eng = nc.sync if ch == 0 \
                                            else nc.scalar
                                        eng.dma_start(
                                            out=out_d[n,
                                                      m * 128:(m + 1) * 128,
                                                      r0:r1],
                                            in_=ob[:, r0:r1, :])
                        else:
                            if j == 2:
                                s12 = epool.tile([128, NQ2, WO], f16,
                                                 name="c6")
                                d12 = epool.tile([128, NQ2, WO], f16,
                                                 name="c7")
                                TT(s12[:, :, :], M[1], M[2], ALU.add)
                                TT(d12[:, :, :], M[1], M[2], ALU.subtract)
                            elif j == 3:
                                TT(ob[:, :, 1, :], d12[:, :, :], M[3],
                                   ALU.subtract)
                            elif j == 0:
                                TT(ob[:, :, 0, :], s12[:, :, :], M[0],
                                   ALU.add)
                                nc.sync.dma_start(
                                    out=out_d[n, m * 128:(m + 1) * 128],
                                    in_=ob[:, :, :, :].rearrange(
                                        "p t f w -> p (t f) w"))
                        if preps:
                            preps.pop(0)()

    nc.compile()
    return nc


def make_in_maps(inputs):
    x = np.ascontiguousarray(np.asarray(inputs["inputVec"], dtype=np.int8))
    w = np.asarray(inputs["weight"], dtype=np.int8)
    scales = np.asarray(inputs["scales"], dtype=np.float32)
    zp = np.asarray(inputs["zeropoints"], dtype=np.int32)
    bias = np.asarray(inputs["bias"], dtype=np.float32)
    assert x.shape == (N, CIN, H, W) and w.shape == (COUT, CIN, KH, KW)

    wq = (w.astype(np.float64) - zp[:, None, None, None]) \
        * (0.01 * scales.astype(np.float64))[:, None, None, None]
    # U[j,o,i,c] = sum_r G[j,r] wq[o,i,r,c]   (transform over row taps)
    U4 = np.einsum("jr,oirc->joic", G4, wq)
    u4 = np.ascontiguousarray(
        U4.reshape(NJ4, MT, 128, KT, 128, KW).transpose(0, 4, 3, 5, 1, 2),
        dtype=np.float16)
    U2 = np.einsum("jr,oirc->joic", G2, wq)
    u2 = np.ascontiguousarray(
        U2.reshape(NJ2, MT, 128, KT, 128, KW).transpose(0, 4, 3, 5, 1, 2),
        dtype=np.float16)
    w1z = (w.astype(np.float64) - zp[:, None, None, None]).sum(axis=(1, 2, 3))
    b2 = (bias.astype(np.float64)
          - 0.07 * scales.astype(np.float64) * w1z).astype(np.float32)
    return [
        {"x": np.ascontiguousarray(x[c * NPER:(c + 1) * NPER]),
         "u4": u4, "u2": u2, "bias2": b2}
        for c in range(NCORES)
    ]


def kernel(**inputs) -> np.ndarray:
    if "nc" not in _CACHE:
        _CACHE["nc"] = _build_program()
    nc = _CACHE["nc"]

    in_maps = make_in_maps(inputs)
    res = run_bass_kernel_spmd(nc, in_maps, list(range(NCORES)))
    out = np.concatenate([res.results[c]["out"] for c in range(NCORES)],
                         axis=0)
    return out


# revision 23
# speedup vs baseline: 1.0711x; 1.0711x over previous
"""Trainium2 Bass kernel for quantized int8 per-channel Conv2d.

Reference semantics (fp32):
  x_f = (x_int8 - 7) * 0.01
  w_f = (w_int8 - zp[cout]) * scale[cout]
  y   = round(conv2d_valid(x_f, w_f) + bias[cout])  -> int32

Algorithm: 1D Winograd along the HEIGHT axis (width taps direct),
ALTERNATING per image between F(2,3) (4 points, cheap transforms) and
F(4,3) (6 points, 2x less PE work, heavier transforms).  The mix
balances the two bottleneck engines: PE ~108us, DVE ~95us per core.
Row-tiling keeps the innermost (width) axis dense so every transform
runs in the DVE's 2x packed fp16 mode with no strided "deal" copies.

F(2,3):  V = [r0-r2, r1+r2, r2-r1, r1-r3]  (27 row-pairs, exact)
         y0 = m0+m1+m2, y1 = m1-m2-m3
F(4,3):  14 quads of 4 rows (input rows up to 57: 2 zero pad rows)
         b0=4(d0-d2)+(d4-d2) b1=-4(d1+d2)+(d3+d4) b2=4(d1-d2)+(d4-d3)
         b3=2e+f b4=-2e+f b5=-4e+(d5-d3)   [e=d3-d1, f=d4-d2]
         A^T=[[1,1,1,1,1,0],[0,1,-1,2,-2,0],[0,1,1,4,4,0],[0,1,-1,8,-8,1]]

U = G*(w-zp)*0.01*scale in fp16; the (x-7) zeropoint folds into the
bias, which rides the j=1 drain (m1's A^T column is all ones).  Output
rounding uses the engines' native fp32->int32 RNE conversion (verified
on HW): the final combines write int32 directly.

Engines: PE j-major matmuls (j=1 first); ACT casts int8->fp16 and
drains PSUM; DVE transforms + A^T combines.  GPSIMD idle (slow SBUF
path + steals the DVE port).  All DMA via sync queue (HWDGE).  Next
image's transform groups are emitted one per combine slot to avoid
bursty DVE queues.  The last (n,m) fuses the j=0 drain+combine (TT
reads PSUM) per chunk.  Sharding: batch 32 over 8 cores.
"""

import numpy as np

import concourse.bass as bass
import concourse.mybir as mybir
from concourse import bacc
from concourse.tile import TileContext
from concourse.bass_utils import run_bass_kernel_spmd

N, CIN, H, W = 32, 256, 56, 56
COUT, KH, KW = 256, 3, 3
HO, WO = H - KH + 1, W - KW + 1          # 54, 54
NCORES = 8
NPER = N // NCORES
HW = H * W
XPAD = HW + 64
KT = CIN // 128
MT = COUT // 128
XR = 60                                   # padded rows in fp16 x buffer

NJ4 = 6                                   # F(4,3) points
NQ4 = 14                                  # row quads
QCH4 = 7                                  # quads per chunk
NCH4 = 2
J4 = (1, 2, 3, 4, 5, 0)

NJ2 = 4                                   # F(2,3) points
NQ2 = 27                                  # row pairs
QCH2 = 9                                  # pairs per chunk
NCH2 = 3
J2 = (1, 2, 3, 0)

KINDS = (2, 4, 2, 4)                      # per-image variant

G4 = np.array([
    [1 / 4, 0, 0],
    [-1 / 6, -1 / 6, -1 / 6],
    [-1 / 6, 1 / 6, -1 / 6],
    [1 / 24, 1 / 12, 1 / 6],
    [1 / 24, -1 / 12, 1 / 6],
    [0, 0, 1],
], dtype=np.float64)
G2 = np.array([
    [1, 0, 0],
    [0.5, 0.5, 0.5],
    [0.5, -0.5, 0.5],
    [0, 0, 1],
], dtype=np.float64)

_CACHE = {}


def _build_program():
    nc = bacc.Bacc("TRN2", target_bir_lowering=False, debug=False,
                   num_devices=NCORES)
    dt = mybir.dt
    f16 = dt.float16
    AF = mybir.ActivationFunctionType
    ALU = mybir.AluOpType

    x_d = nc.dram_tensor("x", [NPER, CIN, H, W], dt.int8, kind="ExternalInput")
    u4_d = nc.dram_tensor("u4", [NJ4, 128, KT, KW, MT, 128], f16,
                          kind="ExternalInput")
    u2_d = nc.dram_tensor("u2", [NJ2, 128, KT, KW, MT, 128], f16,
                          kind="ExternalInput")
    b2_d = nc.dram_tensor("bias2", [COUT], dt.float32, kind="ExternalInput")
    out_d = nc.dram_tensor("out", [NPER, COUT, HO, WO], dt.int32,
                           kind="ExternalOutput")

    with TileContext(nc) as tc:
        with (
            tc.tile_pool(name="const", bufs=1) as cpool,
            tc.tile_pool(name="xin", bufs=2) as xpool,
            tc.tile_pool(name="xf16", bufs=1) as fpool,
            tc.tile_pool(name="v4", bufs=1) as v4pool,
            tc.tile_pool(name="v2", bufs=1) as v2pool,
            tc.tile_pool(name="tsc", bufs=4) as spool,
            tc.tile_pool(name="psum", bufs=7, space="PSUM") as ppool,
            tc.tile_pool(name="m4", bufs=2) as m4pool,
            tc.tile_pool(name="m2", bufs=2) as m2pool,
            tc.tile_pool(name="csc", bufs=1) as epool,
            tc.tile_pool(name="ob4", bufs=1) as o4pool,
            tc.tile_pool(name="ob2", bufs=1) as o2pool,
        ):
            u4sb = cpool.tile([128, NJ4, KT, KW, MT, 128], f16)
            u2sb = cpool.tile([128, NJ2, KT, KW, MT, 128], f16)
            b2 = cpool.tile([128, MT], dt.float32)

            wupw = cpool.tile([128, 128], f16)
            nc.vector.memset(wupw[:, :], 1.0)
            wupx = cpool.tile([128, 512], f16)
            nc.vector.memset(wupx[:, :], 1.0)
            wups = ppool.tile([128, 512], dt.float32, name="wups", tag="wup",
                              bufs=1)
            for _ in range(10):
                nc.tensor.matmul(wups[:, :], wupw[:, :], wupx[:, :],
                                 start=True, stop=True)

            def keepers(rhs, count):
                for _ in range(count):
                    nc.tensor.matmul(wups[:, 0:rhs.shape[-1]], wupw[:, :],
                                     rhs, start=True, stop=True)

            def xdma(n, xb):
                for k in range(KT):
                    nc.sync.dma_start(
                        out=xb[:, k, 0:HW],
                        in_=x_d[n, k * 128:(k + 1) * 128].rearrange(
                            "p h w -> p (h w)"))

            def cast(eng, xb, xf, k):
                dst = xf[:, k].rearrange("p r w -> p (r w)")[:, 0:HW]
                if eng is nc.scalar:
                    eng.copy(dst, xb[:, k, 0:HW])
                else:
                    eng.tensor_copy(dst, xb[:, k, 0:HW])

            TT = nc.vector.tensor_tensor
            STT = nc.vector.scalar_tensor_tensor

            # ---- F(4,3) transforms: merged-k groups (7 emission slots) --
            ts = {}

            def d4v(xf, s):
                xq = xf[:, :, :, :].rearrange("p k (q f) w -> p k q f w",
                                              f=4)
                if s < 4:
                    return xq[:, :, 0:NQ4, s]
                return xq[:, :, 1:NQ4 + 1, s - 4]

            def sc4(name):
                t = spool.tile([128, KT, NQ4, W], f16, name="ts")
                ts[name] = t
                return t[:, :, :, :]

            def g4(name):
                return ts[name][:, :, :, :]

            def prep4(xb, xf, vt):
                d = lambda s: d4v(xf, s)

                def g_cast():
                    nc.vector.memset(xf[:, :, H:H + 2, :], 0.0)
                    cast(nc.scalar, xb, xf, 0)
                    cast(nc.scalar, xb, xf, 1)

                def g_j1():
                    TT(sc4("p1"), d(1), d(2), ALU.add)
                    TT(sc4("p3"), d(3), d(4), ALU.add)
                    STT(vt[:, 1], g4("p1"), -4.0, g4("p3"),
                        ALU.mult, ALU.add)

                def g_j2():
                    TT(sc4("m1"), d(1), d(2), ALU.subtract)
                    TT(sc4("m3"), d(4), d(3), ALU.subtract)
                    STT(vt[:, 2], g4("m1"), 4.0, g4("m3"),
                        ALU.mult, ALU.add)

                def g_j3():
                    TT(sc4("e"), d(3), d(1), ALU.subtract)
                    TT(sc4("f"), d(4), d(2), ALU.subtract)
                    STT(vt[:, 3], g4("e"), 2.0, g4("f"), ALU.mult, ALU.add)

                def g_j4():
                    STT(vt[:, 4], g4("e"), -2.0, g4("f"), ALU.mult, ALU.add)

                def g_j5():
                    TT(sc4("u2"), d(5), d(3), ALU.subtract)
                    STT(vt[:, 5], g4("e"), -4.0, g4("u2"),
                        ALU.mult, ALU.add)

                def g_j0():
                    TT(sc4("u1"), d(0), d(2), ALU.subtract)
                    STT(vt[:, 0], g4("u1"), 4.0, g4("f"), ALU.mult, ALU.add)

                return [g_cast, g_j1, g_j2, g_j3, g_j4, g_j5, g_j0]

            # ---- F(2,3) transforms: merged-k, single-TT points ----------
            def r2v(xf, s):
                xq = xf[:, :, :, :].rearrange("p k (t f) w -> p k t f w",
                                              f=2)
                if s < 2:
                    return xq[:, :, 0:NQ2, s]
                return xq[:, :, 1:NQ2 + 1, s - 2]

            def prep2(xb, xf, vt):
                d = lambda s: r2v(xf, s)

                def g_cast():
                    cast(nc.scalar, xb, xf, 0)
                    cast(nc.scalar, xb, xf, 1)

                def g_j1():
                    TT(vt[:, 1], d(1), d(2), ALU.add)

                def g_j2():
                    TT(vt[:, 2], d(2), d(1), ALU.subtract)

                def g_j3():
                    TT(vt[:, 3], d(1), d(3), ALU.subtract)

                def g_j0():
                    TT(vt[:, 0], d(0), d(2), ALU.subtract)

                return [g_cast, g_j1, g_j2, g_j3, g_j0]

            # ---- startup DMAs: x image 0 first, then U (j=1 first) ------
            xb0 = xpool.tile([128, KT, XPAD], dt.int8, name="xb")
            xf0 = fpool.tile([128, KT, XR, W], f16, name="xf")
            vt20 = v2pool.tile([128, NJ2, KT, NQ2, W], f16, name="vt2")
            xdma(0, xb0)
            nc.sync.dma_start(out=u2sb[:, 1], in_=u2_d[1])
            nc.sync.dma_start(out=b2[:, :],
                              in_=b2_d.rearrange("(m p) -> p m", p=128))
            for j in (2, 3, 0):
                nc.sync.dma_start(out=u2sb[:, j], in_=u2_d[j])
            for j in J4:
                nc.sync.dma_start(out=u4sb[:, j], in_=u4_d[j])

            # ---- image 0 (F23) prologue: split-k for latency ------------
            cast(nc.vector, xb0, xf0, 0)
            cast(nc.scalar, xb0, xf0, 1)
            for j in (1, 2, 3, 0):
                a, b, op = {1: (1, 2, ALU.add), 2: (2, 1, ALU.subtract),
                            3: (1, 3, ALU.subtract),
                            0: (0, 2, ALU.subtract)}[j]
                for k in range(KT):
                    TT(vt20[:, j, k], r2v(xf0, a)[:, k], r2v(xf0, b)[:, k],
                       op)
            keepers(vt20[:, 1, 0].rearrange("p q w -> p (q w)")[:, 0:512],
                    8)

            xfs = {0: xf0}
            vts = {0: vt20}
            preps = []

            for n in range(NPER):
                kind = KINDS[n]
                vt = vts[n]
                last_img = n == NPER - 1
                if not last_img:
                    nkind = KINDS[n + 1]
                    xbn = xpool.tile([128, KT, XPAD], dt.int8, name="xb")
                    xfn = fpool.tile([128, KT, XR, W], f16, name="xf")
                    if nkind == 4:
                        vtn = v4pool.tile([128, NJ4, KT, NQ4, W], f16,
                                          name="vt4")
                        preps = prep4(xbn, xfn, vtn)
                    else:
                        vtn = v2pool.tile([128, NJ2, KT, NQ2, W], f16,
                                          name="vt2")
                        preps = prep2(xbn, xfn, vtn)
                    xdma(n + 1, xbn)
                    xfs[n + 1] = xfn
                    vts[n + 1] = vtn
                else:
                    preps = []

                jorder = J4 if kind == 4 else J2
                nch = NCH4 if kind == 4 else NCH2
                qch = QCH4 if kind == 4 else QCH2
                nq = NQ4 if kind == 4 else NQ2
                usb = u4sb if kind == 4 else u2sb

                for m in range(MT):
                    last = last_img and m == MT - 1
                    if kind == 4:
                        msb = m4pool.tile([128, NJ4, NQ4, WO], f16,
                                          name="msb4")
                        ob = o4pool.tile([128, H, WO], dt.int32, name="ob4")
                        obq = ob[:, :, :].rearrange(
                            "p (q f) w -> p q f w", f=4)
                    else:
                        msb = m2pool.tile([128, NJ2, NQ2, WO], f16,
                                          name="msb2", bufs=1)
                        ob = o2pool.tile([128, NQ2, 2, WO], dt.int32,
                                         name="ob2")
                    M = [msb[:, j] for j in range(len(jorder))]
                    s12 = d12 = s34 = d34 = t0 = u8 = None
                    for j in jorder:
                        ps = [ppool.tile([128, qch, WO], dt.float32,
                                         name="ps", tag="ps")
                              for _ in range(nch)]
                        for k in range(KT):
                            for c in range(KW):
                                lhsT = usb[:, j, k, c, m]
                                for ch in range(nch):
                                    nc.tensor.matmul(
                                        ps[ch][:, :, :], lhsT,
                                        vt[:, j, k, qch * ch:qch * (ch + 1),
                                           c:c + WO],
                                        start=(c == 0 and k == 0),
                                        stop=(c == KW - 1 and k == KT - 1))
                        if not (last and j == 0):
                            for ch in range(nch):
                                dst = msb[:, j, qch * ch:qch * (ch + 1)]
                                if j == 1:
                                    nc.scalar.activation(
                                        dst, ps[ch][:, :, :], AF.Identity,
                                        bias=b2[:, m:m + 1], scale=1.0)
                                else:
                                    nc.scalar.activation(
                                        dst, ps[ch][:, :, :], AF.Copy)
                        # ---- combines (RNE int32 writes) + prep slots ---
                        if kind == 4:
                            if j == 2:
                                s12 = epool.tile([128, NQ4, WO], f16,
                                                 name="c0")
                                d12 = epool.tile([128, NQ4, WO], f16,
                                                 name="c1")
                                TT(s12[:, :, :], M[1], M[2], ALU.add)
                                TT(d12[:, :, :], M[1], M[2], ALU.subtract)
                            elif j == 4:
                                s34 = epool.tile([128, NQ4, WO], f16,
                                                 name="c2")
                                d34 = epool.tile([128, NQ4, WO], f16,
                                                 name="c3")
                                t0 = epool.tile([128, NQ4, WO], f16,
                                                name="c4")
                                u8 = epool.tile([128, NQ4, WO], f16,
                                                name="c5")
                                TT(s34[:, :, :], M[3], M[4], ALU.add)
                                TT(d34[:, :, :], M[3], M[4], ALU.subtract)
                                STT(obq[:, :, 1, :], d34[:, :, :], 2.0,
                                    d12[:, :, :], ALU.mult, ALU.add)
                                STT(obq[:, :, 2, :], s34[:, :, :], 4.0,
                                    s12[:, :, :], ALU.mult, ALU.add)
                                TT(t0[:, :, :], s12[:, :, :], s34[:, :, :],
                                   ALU.add)
                                STT(u8[:, :, :], d34[:, :, :], 8.0,
                                    d12[:, :, :], ALU.mult, ALU.add)
                            elif j == 5:
                                if not last:
                                    TT(obq[:, :, 3, :], u8[:, :, :], M[5],
                                       ALU.add)
                                else:
                                    for ch in range(NCH4):
                                        qs = slice(QCH4 * ch,
                                                   QCH4 * (ch + 1))
                                        TT(obq[:, qs, 3, :], u8[:, qs, :],
                                           msb[:, 5, qs], ALU.add)
                            elif j == 0:
                                if not last:
                                    TT(obq[:, :, 0, :], t0[:, :, :], M[0],
                                       ALU.add)
                                    nc.sync.dma_start(
                                        out=out_d[n, m * 128:(m + 1) * 128],
                                        in_=ob[:, 0:HO, :])
                                else:
                                    for ch in range(NCH4):
                                        qs = slice(QCH4 * ch,
                                                   QCH4 * (ch + 1))
                                        TT(obq[:, qs, 0, :], t0[:, qs, :],
                                           ps[ch][:, :, :], ALU.add)
                                        r0 = 4 * QCH4 * ch
                                        r1 = min(4 * QCH4 * (ch + 1), HO)
                                        eng = nc.sync if ch == 0 \
                                            else nc.scalar
                                        eng.dma_start(
                                            out=out_d[n,
                                                      m * 128:(m + 1) * 128,
                                                      r0:r1],
                                            in_=ob[:, r0:r1, :])
                        else:
                            if j == 2:
                                s12 = epool.tile([128, NQ2, WO], f16,
                                                 name="c6")
                                d12 = epool.tile([128, NQ2, WO], f16,
                                                 name="c7")
                                TT(s12[:, :, :], M[1], M[2], ALU.add)
                                TT(d12[:, :, :], M[1], M[2], ALU.subtract)
                            elif j == 3:
                                TT(ob[:, :, 1, :], d12[:, :, :], M[3],
                                   ALU.subtract)
                            elif j == 0:
                                TT(ob[:, :, 0, :], s12[:, :, :], M[0],
                                   ALU.add)
                                nc.sync.dma_start(
                                    out=out_d[n, m * 128:(m + 1) * 128],
                                    in_=ob[:, :, :, :].rearrange(
                                        "p t f w -> p (t f) w"))
                        if preps:
                            preps.pop(0)()

    nc.compile()
    return nc


def make_in_maps(inputs):
    x = np.ascontiguousarray(np.asarray(inputs["inputVec"], dtype=np.int8))
    w = np.asarray(inputs["weight"], dtype=np.int8)
    scales = np.asarray(inputs["scales"], dtype=np.float32)
    zp = np.asarray(inputs["zeropoints"], dtype=np.int32)
    bias = np.asarray(inputs["bias"], dtype=np.float32)
    assert x.shape == (N, CIN, H, W) and w.shape == (COUT, CIN, KH, KW)

    wq = (w.astype(np.float64) - zp[:, None, None, None]) \
        * (0.01 * scales.astype(np.float64))[:, None, None, None]
    # U[j,o,i,c] = sum_r G[j,r] wq[o,i,r,c]   (transform over row taps)
    U4 = np.einsum("jr,oirc->joic", G4, wq)
    u4 = np.ascontiguousarray(
        U4.reshape(NJ4, MT, 128, KT, 128, KW).transpose(0, 4, 3, 5, 1, 2),
        dtype=np.float16)
    U2 = np.einsum("jr,oirc->joic", G2, wq)
    u2 = np.ascontiguousarray(
        U2.reshape(NJ2, MT, 128, KT, 128, KW).transpose(0, 4, 3, 5, 1, 2),
        dtype=np.float16)
    w1z = (w.astype(np.float64) - zp[:, None, None, None]).sum(axis=(1, 2, 3))
    b2 = (bias.astype(np.float64)
          - 0.07 * scales.astype(np.float64) * w1z).astype(np.float32)
    return [
        {"x": np.ascontiguousarray(x[c * NPER:(c + 1) * NPER]),
         "u4": u4, "u2": u2, "bias2": b2}
        for c in range(NCORES)
    ]


def kernel(**inputs) -> np.ndarray:
    if "nc" not in _CACHE:
        _CACHE["nc"] = _build_program()
    nc = _CACHE["nc"]

    in_maps = make_in_maps(inputs)
    res = run_bass_kernel_spmd(nc, in_maps, list(range(NCORES)))
    out = np.concatenate([res.results[c]["out"] for c in range(NCORES)],
                         axis=0)
    return out


# revision 25
# speedup vs baseline: 1.0838x; 1.0119x over previous
"""Trainium2 Bass kernel for quantized int8 per-channel Conv2d.

Reference semantics (fp32):
  x_f = (x_int8 - 7) * 0.01
  w_f = (w_int8 - zp[cout]) * scale[cout]
  y   = round(conv2d_valid(x_f, w_f) + bias[cout])  -> int32

Algorithm: 1D Winograd along the HEIGHT axis (width taps direct),
ALTERNATING per image between F(2,3) (4 points, cheap transforms) and
F(4,3) (6 points, 2x less PE work, heavier transforms).  The mix
balances the two bottleneck engines: PE ~108us, DVE ~95us per core.
Row-tiling keeps the innermost (width) axis dense so every transform
runs in the DVE's 2x packed fp16 mode with no strided "deal" copies.

F(2,3):  V = [r0-r2, r1+r2, r2-r1, r1-r3]  (27 row-pairs, exact)
         y0 = m0+m1+m2, y1 = m1-m2-m3
F(4,3):  14 quads of 4 rows (input rows up to 57: 2 zero pad rows)
         b0=4(d0-d2)+(d4-d2) b1=-4(d1+d2)+(d3+d4) b2=4(d1-d2)+(d4-d3)
         b3=2e+f b4=-2e+f b5=-4e+(d5-d3)   [e=d3-d1, f=d4-d2]
         A^T=[[1,1,1,1,1,0],[0,1,-1,2,-2,0],[0,1,1,4,4,0],[0,1,-1,8,-8,1]]

U = G*(w-zp)*0.01*scale in fp16; the (x-7) zeropoint folds into the
bias, which rides the j=1 drain (m1's A^T column is all ones).  Output
rounding uses the engines' native fp32->int32 RNE conversion (verified
on HW): the final combines write int32 directly.

Engines: PE j-major matmuls (j=1 first); ACT casts int8->fp16 and
drains PSUM; DVE transforms + A^T combines.  GPSIMD idle (slow SBUF
path + steals the DVE port).  All DMA via sync queue (HWDGE).  Next
image's transform groups are emitted one per combine slot to avoid
bursty DVE queues.  The last (n,m) fuses the j=0 drain+combine (TT
reads PSUM) per chunk.  Sharding: batch 32 over 8 cores.
"""

import numpy as np

import concourse.bass as bass
import concourse.mybir as mybir
from concourse import bacc
from concourse.tile import TileContext
from concourse.bass_utils import run_bass_kernel_spmd

N, CIN, H, W = 32, 256, 56, 56
COUT, KH, KW = 256, 3, 3
HO, WO = H - KH + 1, W - KW + 1          # 54, 54
NCORES = 8
NPER = N // NCORES
HW = H * W
XPAD = HW + 64
KT = CIN // 128
MT = COUT // 128
XR = 60                                   # padded rows in fp16 x buffer

NJ4 = 6                                   # F(4,3) points
NQ4 = 14                                  # row quads
QCH4 = 7                                  # quads per chunk
NCH4 = 2
J4 = (1, 2, 3, 4, 5, 0)

NJ2 = 4                                   # F(2,3) points
NQ2 = 27                                  # row pairs
QCH2 = 9                                  # pairs per chunk
NCH2 = 3
J2 = (1, 2, 3, 0)

KINDS = (2, 4, 2, 4)                      # per-image variant

G4 = np.array([
    [1 / 4, 0, 0],
    [-1 / 6, -1 / 6, -1 / 6],
    [-1 / 6, 1 / 6, -1 / 6],
    [1 / 24, 1 / 12, 1 / 6],
    [1 / 24, -1 / 12, 1 / 6],
    [0, 0, 1],
], dtype=np.float64)
G2 = np.array([
    [1, 0, 0],
    [0.5, 0.5, 0.5],
    [0.5, -0.5, 0.5],
    [0, 0, 1],
], dtype=np.float64)

_CACHE = {}


def _build_program():
    nc = bacc.Bacc("TRN2", target_bir_lowering=False, debug=False,
                   num_devices=NCORES)
    dt = mybir.dt
    f16 = dt.float16
    AF = mybir.ActivationFunctionType
    ALU = mybir.AluOpType

    x_d = nc.dram_tensor("x", [NPER, CIN, H, W], dt.int8, kind="ExternalInput")
    u4_d = nc.dram_tensor("u4", [NJ4, 128, KT, KW, MT, 128], f16,
                          kind="ExternalInput")
    u2_d = nc.dram_tensor("u2", [NJ2, 128, KT, KW, MT, 128], f16,
                          kind="ExternalInput")
    b2_d = nc.dram_tensor("bias2", [COUT], dt.float32, kind="ExternalInput")
    out_d = nc.dram_tensor("out", [NPER, COUT, HO, WO], dt.int32,
                           kind="ExternalOutput")

    with TileContext(nc) as tc:
        with (
            tc.tile_pool(name="const", bufs=1) as cpool,
            tc.tile_pool(name="xin", bufs=2) as xpool,
            tc.tile_pool(name="xf16", bufs=1) as fpool,
            tc.tile_pool(name="v4", bufs=1) as v4pool,
            tc.tile_pool(name="v2", bufs=1) as v2pool,
            tc.tile_pool(name="tsc", bufs=4) as spool,
            tc.tile_pool(name="psum", bufs=7, space="PSUM") as ppool,
            tc.tile_pool(name="m4", bufs=2) as m4pool,
            tc.tile_pool(name="m2", bufs=2) as m2pool,
            tc.tile_pool(name="csc", bufs=1) as epool,
            tc.tile_pool(name="ob4", bufs=1) as o4pool,
            tc.tile_pool(name="ob2", bufs=1) as o2pool,
        ):
            u4sb = cpool.tile([128, NJ4, KT, KW, MT, 128], f16)
            u2sb = cpool.tile([128, NJ2, KT, KW, MT, 128], f16)
            b2 = cpool.tile([128, MT], dt.float32)

            # dummy ACT op at t=0: pulls the ~1.3us ACT_TABLE_LOAD off the
            # image-0 cast critical path
            actw = cpool.tile([128, 8], f16)
            nc.scalar.mul(actw[:, :], actw[:, :], 0.0)

            wupw = cpool.tile([128, 128], f16)
            nc.vector.memset(wupw[:, :], 1.0)
            wupx = cpool.tile([128, 512], f16)
            nc.vector.memset(wupx[:, :], 1.0)
            wups = ppool.tile([128, 512], dt.float32, name="wups", tag="wup",
                              bufs=1)
            for _ in range(10):
                nc.tensor.matmul(wups[:, :], wupw[:, :], wupx[:, :],
                                 start=True, stop=True)

            def keepers(rhs, count):
                for _ in range(count):
                    nc.tensor.matmul(wups[:, 0:rhs.shape[-1]], wupw[:, :],
                                     rhs, start=True, stop=True)

            def xdma(n, xb):
                for k in range(KT):
                    nc.sync.dma_start(
                        out=xb[:, k, 0:HW],
                        in_=x_d[n, k * 128:(k + 1) * 128].rearrange(
                            "p h w -> p (h w)"))

            def cast(eng, xb, xf, k):
                dst = xf[:, k].rearrange("p r w -> p (r w)")[:, 0:HW]
                if eng is nc.scalar:
                    eng.copy(dst, xb[:, k, 0:HW])
                else:
                    eng.tensor_copy(dst, xb[:, k, 0:HW])

            TT = nc.vector.tensor_tensor
            STT = nc.vector.scalar_tensor_tensor

            # ---- F(4,3) transforms: merged-k groups (7 emission slots) --
            ts = {}

            def d4v(xf, s):
                xq = xf[:, :, :, :].rearrange("p k (q f) w -> p k q f w",
                                              f=4)
                if s < 4:
                    return xq[:, :, 0:NQ4, s]
                return xq[:, :, 1:NQ4 + 1, s - 4]

            def sc4(name):
                t = spool.tile([128, KT, NQ4, W], f16, name="ts")
                ts[name] = t
                return t[:, :, :, :]

            def g4(name):
                return ts[name][:, :, :, :]

            def prep4(xb, xf, vt):
                d = lambda s: d4v(xf, s)

                def g_cast():
                    nc.vector.memset(xf[:, :, H:H + 2, :], 0.0)
                    cast(nc.scalar, xb, xf, 0)
                    cast(nc.scalar, xb, xf, 1)

                def g_j1():
                    TT(sc4("p1"), d(1), d(2), ALU.add)
                    TT(sc4("p3"), d(3), d(4), ALU.add)
                    STT(vt[:, 1], g4("p1"), -4.0, g4("p3"),
                        ALU.mult, ALU.add)

                def g_j2():
                    TT(sc4("m1"), d(1), d(2), ALU.subtract)
                    TT(sc4("m3"), d(4), d(3), ALU.subtract)
                    STT(vt[:, 2], g4("m1"), 4.0, g4("m3"),
                        ALU.mult, ALU.add)

                def g_j3():
                    TT(sc4("e"), d(3), d(1), ALU.subtract)
                    TT(sc4("f"), d(4), d(2), ALU.subtract)
                    STT(vt[:, 3], g4("e"), 2.0, g4("f"), ALU.mult, ALU.add)

                def g_j4():
                    STT(vt[:, 4], g4("e"), -2.0, g4("f"), ALU.mult, ALU.add)

                def g_j5():
                    TT(sc4("u2"), d(5), d(3), ALU.subtract)
                    STT(vt[:, 5], g4("e"), -4.0, g4("u2"),
                        ALU.mult, ALU.add)

                def g_j0():
                    TT(sc4("u1"), d(0), d(2), ALU.subtract)
                    STT(vt[:, 0], g4("u1"), 4.0, g4("f"), ALU.mult, ALU.add)

                return [g_cast, g_j1, g_j2, g_j3, g_j4, g_j5, g_j0]

            # ---- F(2,3) transforms: merged-k, single-TT points ----------
            def r2v(xf, s):
                xq = xf[:, :, :, :].rearrange("p k (t f) w -> p k t f w",
                                              f=2)
                if s < 2:
                    return xq[:, :, 0:NQ2, s]
                return xq[:, :, 1:NQ2 + 1, s - 2]

            def prep2(xb, xf, vt):
                d = lambda s: r2v(xf, s)

                def g_cast():
                    cast(nc.scalar, xb, xf, 0)
                    cast(nc.scalar, xb, xf, 1)

                def g_j1():
                    TT(vt[:, 1], d(1), d(2), ALU.add)

                def g_j2():
                    TT(vt[:, 2], d(2), d(1), ALU.subtract)

                def g_j3():
                    TT(vt[:, 3], d(1), d(3), ALU.subtract)

                def g_j0():
                    TT(vt[:, 0], d(0), d(2), ALU.subtract)

                return [g_cast, g_j1, g_j2, g_j3, g_j0]

            # ---- startup DMAs: x image 0 first, then U (j=1 first) ------
            xb0 = xpool.tile([128, KT, XPAD], dt.int8, name="xb")
            xf0 = fpool.tile([128, KT, XR, W], f16, name="xf")
            vt20 = v2pool.tile([128, NJ2, KT, NQ2, W], f16, name="vt2")
            xdma(0, xb0)
            nc.sync.dma_start(out=u2sb[:, 1], in_=u2_d[1])
            nc.sync.dma_start(out=b2[:, :],
                              in_=b2_d.rearrange("(m p) -> p m", p=128))
            for j in (2, 3, 0):
                nc.sync.dma_start(out=u2sb[:, j], in_=u2_d[j])
            for j in J4:
                nc.sync.dma_start(out=u4sb[:, j], in_=u4_d[j])

            # ---- image 0 (F23) prologue: split-k, k0 chain first --------
            cast(nc.vector, xb0, xf0, 0)
            cast(nc.scalar, xb0, xf0, 1)
            JOPS = {1: (1, 2, ALU.add), 2: (2, 1, ALU.subtract),
                    3: (1, 3, ALU.subtract), 0: (0, 2, ALU.subtract)}
            for k in range(KT):
                for j in (1, 2, 3, 0):
                    a, b, op = JOPS[j]
                    TT(vt20[:, j, k], r2v(xf0, a)[:, k], r2v(xf0, b)[:, k],
                       op)
            keepers(vt20[:, 1, 0].rearrange("p q w -> p (q w)")[:, 0:512],
                    8)

            xfs = {0: xf0}
            vts = {0: vt20}
            preps = []

            for n in range(NPER):
                kind = KINDS[n]
                vt = vts[n]
                last_img = n == NPER - 1
                if not last_img:
                    nkind = KINDS[n + 1]
                    xbn = xpool.tile([128, KT, XPAD], dt.int8, name="xb")
                    xfn = fpool.tile([128, KT, XR, W], f16, name="xf")
                    if nkind == 4:
                        vtn = v4pool.tile([128, NJ4, KT, NQ4, W], f16,
                                          name="vt4")
                        preps = prep4(xbn, xfn, vtn)
                    else:
                        vtn = v2pool.tile([128, NJ2, KT, NQ2, W], f16,
                                          name="vt2")
                        preps = prep2(xbn, xfn, vtn)
                    xdma(n + 1, xbn)
                    xfs[n + 1] = xfn
                    vts[n + 1] = vtn
                else:
                    preps = []

                jorder = J4 if kind == 4 else J2
                nch = NCH4 if kind == 4 else NCH2
                qch = QCH4 if kind == 4 else QCH2
                nq = NQ4 if kind == 4 else NQ2
                usb = u4sb if kind == 4 else u2sb

                for m in range(MT):
                    last = last_img and m == MT - 1
                    if kind == 4:
                        msb = m4pool.tile([128, NJ4, NQ4, WO], f16,
                                          name="msb4")
                        ob = o4pool.tile([128, H, WO], dt.int32, name="ob4")
                        obq = ob[:, :, :].rearrange(
                            "p (q f) w -> p q f w", f=4)
                    else:
                        msb = m2pool.tile([128, NJ2, NQ2, WO], f16,
                                          name="msb2", bufs=1)
                        ob = o2pool.tile([128, NQ2, 2, WO], dt.int32,
                                         name="ob2")
                    M = [msb[:, j] for j in range(len(jorder))]
                    s12 = d12 = s34 = d34 = t0 = u8 = None
                    for j in jorder:
                        ps = [ppool.tile([128, qch, WO], dt.float32,
                                         name="ps", tag="ps")
                              for _ in range(nch)]
                        for k in range(KT):
                            for c in range(KW):
                                lhsT = usb[:, j, k, c, m]
                                for ch in range(nch):
                                    nc.tensor.matmul(
                                        ps[ch][:, :, :], lhsT,
                                        vt[:, j, k, qch * ch:qch * (ch + 1),
                                           c:c + WO],
                                        start=(c == 0 and k == 0),
                                        stop=(c == KW - 1 and k == KT - 1))
                        if not (last and j == 0):
                            for ch in range(nch):
                                dst = msb[:, j, qch * ch:qch * (ch + 1)]
                                if j == 1:
                                    nc.scalar.activation(
                                        dst, ps[ch][:, :, :], AF.Identity,
                                        bias=b2[:, m:m + 1], scale=1.0)
                                else:
                                    nc.scalar.activation(
                                        dst, ps[ch][:, :, :], AF.Copy)
                        # ---- combines (RNE int32 writes) + prep slots ---
                        if kind == 4:
                            if j == 2:
                                s12 = epool.tile([128, NQ4, WO], f16,
                                                 name="c0")
                                d12 = epool.tile([128, NQ4, WO], f16,
                                                 name="c1")
                                TT(s12[:, :, :], M[1], M[2], ALU.add)
                                TT(d12[:, :, :], M[1], M[2], ALU.subtract)
                            elif j == 4:
                                s34 = epool.tile([128, NQ4, WO], f16,
                                                 name="c2")
                                d34 = epool.tile([128, NQ4, WO], f16,
                                                 name="c3")
                                t0 = epool.tile([128, NQ4, WO], f16,
                                                name="c4")
                                u8 = epool.tile([128, NQ4, WO], f16,
                                                name="c5")
                                TT(s34[:, :, :], M[3], M[4], ALU.add)
                                TT(d34[:, :, :], M[3], M[4], ALU.subtract)
                                STT(obq[:, :, 1, :], d34[:, :, :], 2.0,
                                    d12[:, :, :], ALU.mult, ALU.add)
                                STT(obq[:, :, 2, :], s34[:, :, :], 4.0,
                                    s12[:, :, :], ALU.mult, ALU.add)
                                TT(t0[:, :, :], s12[:, :, :], s34[:, :, :],
                                   ALU.add)
                                STT(u8[:, :, :], d34[:, :, :], 8.0,
                                    d12[:, :, :], ALU.mult, ALU.add)
                            elif j == 5:
                                if not last:
                                    TT(obq[:, :, 3, :], u8[:, :, :], M[5],
                                       ALU.add)
                                else:
                                    for ch in range(NCH4):
                                        qs = slice(QCH4 * ch,
                                                   QCH4 * (ch + 1))
                                        TT(obq[:, qs, 3, :], u8[:, qs, :],
                                           msb[:, 5, qs], ALU.add)
                            elif j == 0:
                                if not last:
                                    TT(obq[:, :, 0, :], t0[:, :, :], M[0],
                                       ALU.add)
                                    nc.sync.dma_start(
                                        out=out_d[n, m * 128:(m + 1) * 128],
                                        in_=ob[:, 0:HO, :])
                                else:
                                    for ch in range(NCH4):
                                        qs = slice(QCH4 * ch,
                                                   QCH4 * (ch + 1))
                                        TT(obq[:, qs, 0, :], t0[:, qs, :],
                                           ps[ch][:, :, :], ALU.add)
                                        r0 = 4 * QCH4 * ch
                                        r1 = min(4 * QCH4 * (ch + 1), HO)
                                        eng = nc.sync if ch == 0 \
                                            else nc.scalar
                                        eng.dma_start(
                                            out=out_d[n,
                                                      m * 128:(m + 1) * 128,
                                                      r0:r1],
                                            in_=ob[:, r0:r1, :])
                        else:
                            if j == 2:
                                s12 = epool.tile([128, NQ2, WO], f16,
                                                 name="c6")
                                d12 = epool.tile([128, NQ2, WO], f16,
                                                 name="c7")
                                TT(s12[:, :, :], M[1], M[2], ALU.add)
                                TT(d12[:, :, :], M[1], M[2], ALU.subtract)
                            elif j == 3:
                                TT(ob[:, :, 1, :], d12[:, :, :], M[3],
                                   ALU.subtract)
                            elif j == 0:
                                TT(ob[:, :, 0, :], s12[:, :, :], M[0],
                                   ALU.add)
                                nc.sync.dma_start(
                                    out=out_d[n, m * 128:(m + 1) * 128],
                                    in_=ob[:, :, :, :].rearrange(
                                        "p t f w -> p (t f) w"))
                        if preps:
                            preps.pop(0)()

    nc.compile()
    return nc


def make_in_maps(inputs):
    x = np.ascontiguousarray(np.asarray(inputs["inputVec"], dtype=np.int8))
    w = np.asarray(inputs["weight"], dtype=np.int8)
    scales = np.asarray(inputs["scales"], dtype=np.float32)
    zp = np.asarray(inputs["zeropoints"], dtype=np.int32)
    bias = np.asarray(inputs["bias"], dtype=np.float32)
    assert x.shape == (N, CIN, H, W) and w.shape == (COUT, CIN, KH, KW)

    wq = (w.astype(np.float64) - zp[:, None, None, None]) \
        * (0.01 * scales.astype(np.float64))[:, None, None, None]
    # U[j,o,i,c] = sum_r G[j,r] wq[o,i,r,c]   (transform over row taps)
    U4 = np.einsum("jr,oirc->joic", G4, wq)
    u4 = np.ascontiguousarray(
        U4.reshape(NJ4, MT, 128, KT, 128, KW).transpose(0, 4, 3, 5, 1, 2),
        dtype=np.float16)
    U2 = np.einsum("jr,oirc->joic", G2, wq)
    u2 = np.ascontiguousarray(
        U2.reshape(NJ2, MT, 128, KT, 128, KW).transpose(0, 4, 3, 5, 1, 2),
        dtype=np.float16)
    w1z = (w.astype(np.float64) - zp[:, None, None, None]).sum(axis=(1, 2, 3))
    b2 = (bias.astype(np.float64)
          - 0.07 * scales.astype(np.float64) * w1z).astype(np.float32)
    return [
        {"x": np.ascontiguousarray(x[c * NPER:(c + 1) * NPER]),
         "u4": u4, "u2": u2, "bias2": b2}
        for c in range(NCORES)
    ]


def kernel(**inputs) -> np.ndarray:
    if "nc" not in _CACHE:
        _CACHE["nc"] = _build_program()
    nc = _CACHE["nc"]

    in_maps = make_in_maps(inputs)
    res = run_bass_kernel_spmd(nc, in_maps, list(range(NCORES)))
    out = np.concatenate([res.results[c]["out"] for c in range(NCORES)],
                         axis=0)
    return out


# revision 30
# speedup vs baseline: 1.0866x; 1.0026x over previous
"""Trainium2 Bass kernel for quantized int8 per-channel Conv2d.

Reference semantics (fp32):
  x_f = (x_int8 - 7) * 0.01
  w_f = (w_int8 - zp[cout]) * scale[cout]
  y   = round(conv2d_valid(x_f, w_f) + bias[cout])  -> int32

Algorithm: 1D Winograd along the HEIGHT axis (width taps direct),
ALTERNATING per image between F(2,3) (4 points, cheap transforms) and
F(4,3) (6 points, 2x less PE work, heavier transforms).  The mix
balances the two bottleneck engines: PE ~108us, DVE ~95us per core.
Row-tiling keeps the innermost (width) axis dense so every transform
runs in the DVE's 2x packed fp16 mode with no strided "deal" copies.

F(2,3):  V = [r0-r2, r1+r2, r2-r1, r1-r3]  (27 row-pairs, exact)
         y0 = m0+m1+m2, y1 = m1-m2-m3
F(4,3):  14 quads of 4 rows (input rows up to 57: 2 zero pad rows)
         b0=4(d0-d2)+(d4-d2) b1=-4(d1+d2)+(d3+d4) b2=4(d1-d2)+(d4-d3)
         b3=2e+f b4=-2e+f b5=-4e+(d5-d3)   [e=d3-d1, f=d4-d2]
         A^T=[[1,1,1,1,1,0],[0,1,-1,2,-2,0],[0,1,1,4,4,0],[0,1,-1,8,-8,1]]

U = G*(w-zp)*0.01*scale in fp16; the (x-7) zeropoint folds into the
bias, which rides the j=1 drain (m1's A^T column is all ones).  Output
rounding uses the engines' native fp32->int32 RNE conversion (verified
on HW): the final combines write int32 directly.

Engines: PE j-major matmuls (j=1 first); ACT casts int8->fp16 and
drains PSUM; DVE transforms + A^T combines.  GPSIMD idle (slow SBUF
path + steals the DVE port).  All DMA via sync queue (HWDGE).  Next
image's transform groups are emitted one per combine slot to avoid
bursty DVE queues.  The last (n,m) fuses the j=0 drain+combine (TT
reads PSUM) per chunk.  Sharding: batch 32 over 8 cores.
"""

import numpy as np

import concourse.bass as bass
import concourse.mybir as mybir
from concourse import bacc
from concourse.tile import TileContext
from concourse.bass_utils import run_bass_kernel_spmd

N, CIN, H, W = 32, 256, 56, 56
COUT, KH, KW = 256, 3, 3
HO, WO = H - KH + 1, W - KW + 1          # 54, 54
NCORES = 8
NPER = N // NCORES
HW = H * W
XPAD = HW + 64
KT = CIN // 128
MT = COUT // 128
XR = 60                                   # padded rows in fp16 x buffer

NJ4 = 6                                   # F(4,3) points
NQ4 = 14                                  # row quads
QCH4 = 7                                  # quads per chunk
NCH4 = 2
J4 = (1, 2, 3, 4, 5, 0)

NJ2 = 4                                   # F(2,3) points
NQ2 = 27                                  # row pairs
QCH2 = 9                                  # pairs per chunk
NCH2 = 3
J2 = (1, 2, 3, 0)

KINDS = (2, 4, 2, 4)                      # per-image variant

G4 = np.array([
    [1 / 4, 0, 0],
    [-1 / 6, -1 / 6, -1 / 6],
    [-1 / 6, 1 / 6, -1 / 6],
    [1 / 24, 1 / 12, 1 / 6],
    [1 / 24, -1 / 12, 1 / 6],
    [0, 0, 1],
], dtype=np.float64)
G2 = np.array([
    [1, 0, 0],
    [0.5, 0.5, 0.5],
    [0.5, -0.5, 0.5],
    [0, 0, 1],
], dtype=np.float64)

_CACHE = {}


def _build_program():
    nc = bacc.Bacc("TRN2", target_bir_lowering=False, debug=False,
                   num_devices=NCORES)
    dt = mybir.dt
    f16 = dt.float16
    AF = mybir.ActivationFunctionType
    ALU = mybir.AluOpType

    x_d = nc.dram_tensor("x", [NPER, CIN, H, W], dt.int8, kind="ExternalInput")
    u4_d = nc.dram_tensor("u4", [NJ4, 128, KT, KW, MT, 128], f16,
                          kind="ExternalInput")
    u2_d = nc.dram_tensor("u2", [NJ2, 128, KT, KW, MT, 128], f16,
                          kind="ExternalInput")
    b2_d = nc.dram_tensor("bias2", [COUT], dt.float32, kind="ExternalInput")
    out_d = nc.dram_tensor("out", [NPER, COUT, HO, WO], dt.int32,
                           kind="ExternalOutput")

    with TileContext(nc) as tc:
        with (
            tc.tile_pool(name="const", bufs=1) as cpool,
            tc.tile_pool(name="xin", bufs=2) as xpool,
            tc.tile_pool(name="xf16", bufs=1) as fpool,
            tc.tile_pool(name="v4", bufs=1) as v4pool,
            tc.tile_pool(name="v2", bufs=1) as v2pool,
            tc.tile_pool(name="tsc", bufs=4) as spool,
            tc.tile_pool(name="psum", bufs=8, space="PSUM") as ppool,
            tc.tile_pool(name="m4", bufs=2) as m4pool,
            tc.tile_pool(name="m2", bufs=2) as m2pool,
            tc.tile_pool(name="csc", bufs=1) as epool,
            tc.tile_pool(name="ob4", bufs=1) as o4pool,
            tc.tile_pool(name="ob2", bufs=1) as o2pool,
        ):
            u4sb = cpool.tile([128, NJ4, KT, KW, MT, 128], f16)
            u2sb = cpool.tile([128, NJ2, KT, KW, MT, 128], f16)
            b2 = cpool.tile([128, MT], dt.float32)

            wupw = cpool.tile([128, 128], f16)
            nc.vector.memset(wupw[:, :], 1.0)
            wupx = cpool.tile([128, 486], f16)
            nc.vector.memset(wupx[:, :], 1.0)

            # warmups write rotating ps-pool tiles: no standing PSUM bank,
            # so the full 8 banks are free for the pipeline afterwards
            def wmm(rhs, count):
                for _ in range(count):
                    ws = ppool.tile([128, 486], dt.float32, name="ps",
                                    tag="ps")
                    nc.tensor.matmul(ws[:, 0:rhs.shape[-1]], wupw[:, :],
                                     rhs, start=True, stop=True)

            def keepers(rhs, count):
                wmm(rhs, count)

            def xdma(n, xb):
                for k in range(KT):
                    nc.sync.dma_start(
                        out=xb[:, k, 0:HW],
                        in_=x_d[n, k * 128:(k + 1) * 128].rearrange(
                            "p h w -> p (h w)"))

            def cast(eng, xb, xf, k):
                dst = xf[:, k].rearrange("p r w -> p (r w)")[:, 0:HW]
                if eng is nc.scalar:
                    eng.copy(dst, xb[:, k, 0:HW])
                else:
                    eng.tensor_copy(dst, xb[:, k, 0:HW])

            TT = nc.vector.tensor_tensor
            STT = nc.vector.scalar_tensor_tensor

            # ---- F(4,3) transforms: merged-k groups (7 emission slots) --
            ts = {}

            def d4v(xf, s):
                xq = xf[:, :, :, :].rearrange("p k (q f) w -> p k q f w",
                                              f=4)
                if s < 4:
                    return xq[:, :, 0:NQ4, s]
                return xq[:, :, 1:NQ4 + 1, s - 4]

            def sc4(name):
                t = spool.tile([128, KT, NQ4, W], f16, name="ts")
                ts[name] = t
                return t[:, :, :, :]

            def g4(name):
                return ts[name][:, :, :, :]

            def prep4(xb, xf, vt):
                d = lambda s: d4v(xf, s)

                def g_cast():
                    nc.vector.memset(xf[:, :, H:H + 2, :], 0.0)
                    cast(nc.scalar, xb, xf, 0)
                    cast(nc.scalar, xb, xf, 1)

                def g_j1():
                    TT(sc4("p1"), d(1), d(2), ALU.add)
                    TT(sc4("p3"), d(3), d(4), ALU.add)
                    STT(vt[:, 1], g4("p1"), -4.0, g4("p3"),
                        ALU.mult, ALU.add)

                def g_j2():
                    TT(sc4("m1"), d(1), d(2), ALU.subtract)
                    TT(sc4("m3"), d(4), d(3), ALU.subtract)
                    STT(vt[:, 2], g4("m1"), 4.0, g4("m3"),
                        ALU.mult, ALU.add)

                def g_j3():
                    TT(sc4("e"), d(3), d(1), ALU.subtract)
                    TT(sc4("f"), d(4), d(2), ALU.subtract)
                    STT(vt[:, 3], g4("e"), 2.0, g4("f"), ALU.mult, ALU.add)

                def g_j4():
                    STT(vt[:, 4], g4("e"), -2.0, g4("f"), ALU.mult, ALU.add)

                def g_j5():
                    TT(sc4("u2"), d(5), d(3), ALU.subtract)
                    STT(vt[:, 5], g4("e"), -4.0, g4("u2"),
                        ALU.mult, ALU.add)

                def g_j0():
                    TT(sc4("u1"), d(0), d(2), ALU.subtract)
                    STT(vt[:, 0], g4("u1"), 4.0, g4("f"), ALU.mult, ALU.add)

                return [g_cast, g_j1, g_j2, g_j3, g_j4, g_j5, g_j0]

            # ---- F(2,3) transforms: merged-k, single-TT points ----------
            def r2v(xf, s):
                xq = xf[:, :, :, :].rearrange("p k (t f) w -> p k t f w",
                                              f=2)
                if s < 2:
                    return xq[:, :, 0:NQ2, s]
                return xq[:, :, 1:NQ2 + 1, s - 2]

            def prep2(xb, xf, vt):
                d = lambda s: r2v(xf, s)

                def g_cast():
                    cast(nc.scalar, xb, xf, 0)
                    cast(nc.scalar, xb, xf, 1)

                def g_j1():
                    TT(vt[:, 1], d(1), d(2), ALU.add)

                def g_j2():
                    TT(vt[:, 2], d(2), d(1), ALU.subtract)

                def g_j3():
                    TT(vt[:, 3], d(1), d(3), ALU.subtract)

                def g_j0():
                    TT(vt[:, 0], d(0), d(2), ALU.subtract)

                return [g_cast, g_j1, g_j2, g_j3, g_j0]

            # ---- startup DMAs: x image 0 first, then U (j=1 first) ------
            xb0 = xpool.tile([128, KT, XPAD], dt.int8, name="xb")
            xf0 = fpool.tile([128, KT, XR, W], f16, name="xf")
            vt20 = v2pool.tile([128, NJ2, KT, NQ2, W], f16, name="vt2")
            # image-0 x: k0 on sync, k1 on the scalar HWDGE queue (parallel)
            nc.sync.dma_start(out=xb0[:, 0, 0:HW],
                              in_=x_d[0, 0:128].rearrange("p h w -> p (h w)"))
            nc.scalar.dma_start(out=xb0[:, 1, 0:HW],
                                in_=x_d[0, 128:256].rearrange(
                                    "p h w -> p (h w)"))
            # dummy ACT op: pulls the ~1.3us ACT_TABLE_LOAD off the
            # image-0 cast critical path (queued behind the DMA issue)
            actw = cpool.tile([128, 8], f16)
            nc.scalar.mul(actw[:, :], actw[:, :], 0.0)
            wmm(wupx[:, :], 10)
            nc.sync.dma_start(out=u2sb[:, 1], in_=u2_d[1])
            nc.sync.dma_start(out=b2[:, :],
                              in_=b2_d.rearrange("(m p) -> p m", p=128))
            for j in (2, 3, 0):
                nc.sync.dma_start(out=u2sb[:, j], in_=u2_d[j])
            for j in J4:
                nc.sync.dma_start(out=u4sb[:, j], in_=u4_d[j])

            # ---- image 0 (F23) prologue: split-k, k0 chain first --------
            cast(nc.vector, xb0, xf0, 0)
            cast(nc.scalar, xb0, xf0, 1)
            JOPS = {1: (1, 2, ALU.add), 2: (2, 1, ALU.subtract),
                    3: (1, 3, ALU.subtract), 0: (0, 2, ALU.subtract)}
            for k in range(KT):
                for j in (1, 2, 3, 0):
                    a, b, op = JOPS[j]
                    TT(vt20[:, j, k], r2v(xf0, a)[:, k], r2v(xf0, b)[:, k],
                       op)
            keepers(vt20[:, 1, 0].rearrange("p q w -> p (q w)")[:, 0:486],
                    8)

            xfs = {0: xf0}
            vts = {0: vt20}
            preps = []

            for n in range(NPER):
                kind = KINDS[n]
                vt = vts[n]
                last_img = n == NPER - 1
                if not last_img:
                    nkind = KINDS[n + 1]
                    xbn = xpool.tile([128, KT, XPAD], dt.int8, name="xb")
                    xfn = fpool.tile([128, KT, XR, W], f16, name="xf")
                    if nkind == 4:
                        vtn = v4pool.tile([128, NJ4, KT, NQ4, W], f16,
                                          name="vt4")
                        preps = prep4(xbn, xfn, vtn)
                    else:
                        vtn = v2pool.tile([128, NJ2, KT, NQ2, W], f16,
                                          name="vt2")
                        preps = prep2(xbn, xfn, vtn)
                    xdma(n + 1, xbn)
                    xfs[n + 1] = xfn
                    vts[n + 1] = vtn
                else:
                    preps = []

                jorder = J4 if kind == 4 else J2
                nch = NCH4 if kind == 4 else NCH2
                qch = QCH4 if kind == 4 else QCH2
                nq = NQ4 if kind == 4 else NQ2
                usb = u4sb if kind == 4 else u2sb

                for m in range(MT):
                    last = last_img and m == MT - 1
                    if kind == 4:
                        msb = m4pool.tile([128, NJ4, NQ4, WO], f16,
                                          name="msb4")
                        ob = o4pool.tile([128, H, WO], dt.int32, name="ob4")
                        obq = ob[:, :, :].rearrange(
                            "p (q f) w -> p q f w", f=4)
                    else:
                        msb = m2pool.tile([128, NJ2, NQ2, WO], f16,
                                          name="msb2", bufs=1)
                        ob = o2pool.tile([128, NQ2, 2, WO], dt.int32,
                                         name="ob2")
                    M = [msb[:, j] for j in range(len(jorder))]
                    s12 = d12 = s34 = d34 = t0 = u8 = None
                    for j in jorder:
                        ps = [ppool.tile([128, qch, WO], dt.float32,
                                         name="ps", tag="ps")
                              for _ in range(nch)]
                        for k in range(KT):
                            for c in range(KW):
                                lhsT = usb[:, j, k, c, m]
                                for ch in range(nch):
                                    nc.tensor.matmul(
                                        ps[ch][:, :, :], lhsT,
                                        vt[:, j, k, qch * ch:qch * (ch + 1),
                                           c:c + WO],
                                        start=(c == 0 and k == 0),
                                        stop=(c == KW - 1 and k == KT - 1))
                        if not (last and j == 0):
                            for ch in range(nch):
                                dst = msb[:, j, qch * ch:qch * (ch + 1)]
                                if j == 1:
                                    nc.scalar.activation(
                                        dst, ps[ch][:, :, :], AF.Identity,
                                        bias=b2[:, m:m + 1], scale=1.0)
                                else:
                                    nc.scalar.activation(
                                        dst, ps[ch][:, :, :], AF.Copy)
                        # ---- combines (RNE int32 writes) + prep slots ---
                        if kind == 4:
                            if j == 2:
                                s12 = epool.tile([128, NQ4, WO], f16,
                                                 name="c0")
                                d12 = epool.tile([128, NQ4, WO], f16,
                                                 name="c1")
                                TT(s12[:, :, :], M[1], M[2], ALU.add)
                                TT(d12[:, :, :], M[1], M[2], ALU.subtract)
                            elif j == 4:
                                s34 = epool.tile([128, NQ4, WO], f16,
                                                 name="c2")
                                d34 = epool.tile([128, NQ4, WO], f16,
                                                 name="c3")
                                t0 = epool.tile([128, NQ4, WO], f16,
                                                name="c4")
                                u8 = epool.tile([128, NQ4, WO], f16,
                                                name="c5")
                                TT(s34[:, :, :], M[3], M[4], ALU.add)
                                TT(d34[:, :, :], M[3], M[4], ALU.subtract)
                                if not last:
                                    STT(obq[:, :, 1, :], d34[:, :, :], 2.0,
                                        d12[:, :, :], ALU.mult, ALU.add)
                                    STT(obq[:, :, 2, :], s34[:, :, :], 4.0,
                                        s12[:, :, :], ALU.mult, ALU.add)
                                    TT(t0[:, :, :], s12[:, :, :],
                                       s34[:, :, :], ALU.add)
                                    STT(u8[:, :, :], d34[:, :, :], 8.0,
                                        d12[:, :, :], ALU.mult, ALU.add)
                                else:
                                    # tail: y3/y0 prerequisites first, the
                                    # independent y1/y2 per chunk after
                                    TT(t0[:, :, :], s12[:, :, :],
                                       s34[:, :, :], ALU.add)
                                    STT(u8[:, :, :], d34[:, :, :], 8.0,
                                        d12[:, :, :], ALU.mult, ALU.add)
                                    for ch in range(NCH4):
                                        qs = slice(QCH4 * ch,
                                                   QCH4 * (ch + 1))
                                        STT(obq[:, qs, 1, :],
                                            d34[:, qs, :], 2.0,
                                            d12[:, qs, :],
                                            ALU.mult, ALU.add)
                                        STT(obq[:, qs, 2, :],
                                            s34[:, qs, :], 4.0,
                                            s12[:, qs, :],
                                            ALU.mult, ALU.add)
                            elif j == 5:
                                if not last:
                                    TT(obq[:, :, 3, :], u8[:, :, :], M[5],
                                       ALU.add)
                                else:
                                    for ch in range(NCH4):
                                        qs = slice(QCH4 * ch,
                                                   QCH4 * (ch + 1))
                                        TT(obq[:, qs, 3, :], u8[:, qs, :],
                                           msb[:, 5, qs], ALU.add)
                            elif j == 0:
                                if not last:
                                    TT(obq[:, :, 0, :], t0[:, :, :], M[0],
                                       ALU.add)
                                    nc.sync.dma_start(
                                        out=out_d[n, m * 128:(m + 1) * 128],
                                        in_=ob[:, 0:HO, :])
                                else:
                                    for ch in range(NCH4):
                                        qs = slice(QCH4 * ch,
                                                   QCH4 * (ch + 1))
                                        TT(obq[:, qs, 0, :], t0[:, qs, :],
                                           ps[ch][:, :, :], ALU.add)
                                        r0 = 4 * QCH4 * ch
                                        r1 = min(4 * QCH4 * (ch + 1), HO)
                                        eng = nc.sync if ch == 0 \
                                            else nc.scalar
                                        eng.dma_start(
                                            out=out_d[n,
                                                      m * 128:(m + 1) * 128,
                                                      r0:r1],
                                            in_=ob[:, r0:r1, :])
                        else:
                            if j == 2:
                                s12 = epool.tile([128, NQ2, WO], f16,
                                                 name="c6")
                                d12 = epool.tile([128, NQ2, WO], f16,
                                                 name="c7")
                                TT(s12[:, :, :], M[1], M[2], ALU.add)
                                TT(d12[:, :, :], M[1], M[2], ALU.subtract)
                            elif j == 3:
                                TT(ob[:, :, 1, :], d12[:, :, :], M[3],
                                   ALU.subtract)
                            elif j == 0:
                                TT(ob[:, :, 0, :], s12[:, :, :], M[0],
                                   ALU.add)
                                nc.sync.dma_start(
                                    out=out_d[n, m * 128:(m + 1) * 128],
                                    in_=ob[:, :, :, :].rearrange(
                                        "p t f w -> p (t f) w"))
                        if preps:
                            preps.pop(0)()

    nc.compile()
    return nc


def make_in_maps(inputs):
    x = np.ascontiguousarray(np.asarray(inputs["inputVec"], dtype=np.int8))
    w = np.asarray(inputs["weight"], dtype=np.int8)
    scales = np.asarray(inputs["scales"], dtype=np.float32)
    zp = np.asarray(inputs["zeropoints"], dtype=np.int32)
    bias = np.asarray(inputs["bias"], dtype=np.float32)
    assert x.shape == (N, CIN, H, W) and w.shape == (COUT, CIN, KH, KW)

    wq = (w.astype(np.float64) - zp[:, None, None, None]) \
        * (0.01 * scales.astype(np.float64))[:, None, None, None]
    # U[j,o,i,c] = sum_r G[j,r] wq[o,i,r,c]   (transform over row taps)
    U4 = np.einsum("jr,oirc->joic", G4, wq)
    u4 = np.ascontiguousarray(
        U4.reshape(NJ4, MT, 128, KT, 128, KW).transpose(0, 4, 3, 5, 1, 2),
        dtype=np.float16)
    U2 = np.einsum("jr,oirc->joic", G2, wq)
    u2 = np.ascontiguousarray(
        U2.reshape(NJ2, MT, 128, KT, 128, KW).transpose(0, 4, 3, 5, 1, 2),
        dtype=np.float16)
    w1z = (w.astype(np.float64) - zp[:, None, None, None]).sum(axis=(1, 2, 3))
    b2 = (bias.astype(np.float64)
          - 0.07 * scales.astype(np.float64) * w1z).astype(np.float32)
    return [
        {"x": np.ascontiguousarray(x[c * NPER:(c + 1) * NPER]),
         "u4": u4, "u2": u2, "bias2": b2}
        for c in range(NCORES)
    ]


def kernel(**inputs) -> np.ndarray:
    if "nc" not in _CACHE:
        _CACHE["nc"] = _build_program()
    nc = _CACHE["nc"]

    in_maps = make_in_maps(inputs)
    res = run_bass_kernel_spmd(nc, in_maps, list(range(NCORES)))
    out = np.concatenate([res.results[c]["out"] for c in range(NCORES)],
                         axis=0)
    return out
